# revision 28
# baseline (speedup 1.0000x reference)
"""Bass/Tile kernel for nn_AligningModel on 8 Trainium2 NeuronCores.

Data parallel: 32 samples sharded 4-per-core across 8 cores; all params
replicated.  Inside each core the model runs feature-major ([C, T] layout)
with bf16/fp8 matmul operands and fp32 PSUM accumulation:

  mel GLU encoder (4 layers)  -> mel_enc   [384, T]
  phoneme GLU encoder (4)     -> ph_enc    [384, 257]
  -L2^2 attention softmax     -> context   [384, T]   (the |mel|^2 term
      cancels inside softmax, so scores are 2*mel.ph - |ph|^2 - penalty)
  dec GLU (2 layers, 768ch)   -> dec       [768, T]
  logits twice (token-major for log_softmax, feature-major for mel decoder)
  mel GLU decoder (2 layers, 128ch) -> tanh mel preds

Convolutions (k=3, SAME) are shifted matmuls accumulated in PSUM.  Conv
halves selected in QCONF run in fp8-e4m3 DoubleRow: weights are packed
[rows, 3*NC, Co] so block pairs ((c,k0),(c,k2)) and ((c0,k1),(c1,k1)) form
DR passes whose moving APs read straight out of one packed fp8 activation
tile (middle-dim strides 2 and XMW — no interleave copies).  Per-cout
weight scales fold into the sigmoid scale / bias / residual multiplier.
"""

import numpy as np
import ml_dtypes

BF = ml_dtypes.bfloat16
F8 = ml_dtypes.float8_e4m3   # TRN fp8_e4m3 (max normal 240)

B_FULL = 32
T_MEL = 2000
T_PHON = 256
E = 384
V = 256
MEL_DIMS = 80
DEC_H = 128
N_CORES = 8
NS = B_FULL // N_CORES  # samples per core

# which conv halves run fp8-DoubleRow, per group and layer (mel L0 fused,
# mel enc L1-3 [slot 0 unused], phoneme enc, decoder, mel decoder)
QCONF = {
    'me0': {'a': [False], 'g': [False]},
    'me': {'a': [False] * 4, 'g': [False, True, True, False]},
    'pe': {'a': [False] * 4, 'g': [True] * 4},
    'pd': {'a': [False] * 2, 'g': [True] * 2},
    'md': {'a': [True] * 2, 'g': [True] * 2},
}


def _ttiles(T, w=512):
    out, t0 = [], 0
    while t0 < T:
        out.append((t0, min(w, T - t0)))
        t0 += w
    return out


def build(ns=NS, T=T_MEL, TPH=T_PHON, teffs=None):
    """Build and compile the per-core Bass kernel (ns samples, seq len T)."""
    import concourse.bacc as bacc
    import concourse.tile as tile
    import concourse.mybir as mybir
    from concourse.alu_op_type import AluOpType as aop
    from concourse.masks import make_identity
    import concourse.bass as bass

    f32 = mybir.dt.float32
    bf16 = mybir.dt.bfloat16
    f8e4 = mybir.dt.float8e4
    DR = mybir.MatmulPerfMode.DoubleRow
    AF = mybir.ActivationFunctionType
    AX = mybir.AxisListType

    if teffs is None:
        teffs = [T] * ns
    assert len(teffs) == ns and all(128 <= t <= T and t % 128 == 0 or t == T
                                    for t in teffs)
    S = TPH + 1          # phoneme positions incl. prepended blank
    TP = T + 2           # zero border column each side
    SP = S + 2
    TT = _ttiles(T)          # 512-wide t tiles
    TBLK = _ttiles(T, 128)   # 128-wide t blocks
    SBLK = _ttiles(S, 128)   # s chunks
    EB = E // 128            # 3
    HB = 2 * E // 128        # 6
    VB = V // 128            # 2
    XMW = 516                # xm chunk slot width (512 + 2 halo + margin)

    nc = bacc.Bacc("TRN2", debug=False, target_bir_lowering=False)

    def wdt(g, h, l):
        return f8e4 if QCONF[g][h][l] else bf16

    # ---------------- DRAM I/O ----------------
    d_mels = nc.dram_tensor("mels", [ns, MEL_DIMS + 1, T], bf16, kind="ExternalInput").ap()
    d_embph = nc.dram_tensor("embph", [ns, E, SP], bf16, kind="ExternalInput").ap()
    d_melmask = nc.dram_tensor("melmask", [ns, TP], bf16, kind="ExternalInput").ap()
    d_phmask = nc.dram_tensor("phmask", [ns, SP], bf16, kind="ExternalInput").ap()
    d_phpen = nc.dram_tensor("phpen", [ns, S], f32, kind="ExternalInput").ap()

    d_meproj81 = nc.dram_tensor("me_proj81", [MEL_DIMS + 1, E], bf16, kind="ExternalInput").ap()
    # packed conv weights, one tensor per layer: [rows, 3*NC, Co], b = c*3 + k
    GSHAPE = {'me0': (1, MEL_DIMS + 1, 3, E), 'me': (4, 128, 9, E),
              'pe': (4, 128, 9, E), 'pd': (2, 128, 18, 2 * E),
              'md': (2, 128, 3, DEC_H)}
    d_w = {}
    for g, (L, R, NB, Co) in GSHAPE.items():
        for h in ('a', 'g'):
            for l in range(L):
                if g == 'me' and l == 0:
                    continue
                d_w[(g, h, l)] = nc.dram_tensor(
                    f"w_{g}_{h}_{l}", [R, NB, Co], wdt(g, h, l),
                    kind="ExternalInput").ap()
    # bias/scale tables, host-packed partition-major: [128, L, 4, Co//128]
    d_tab = {}
    for g, L, Co in (('me', 4, E), ('pe', 4, E), ('pd', 2, 2 * E), ('md', 2, DEC_H)):
        d_tab[g] = nc.dram_tensor(f"tab_{g}", [128, L, 4, Co // 128], f32,
                                  kind="ExternalInput").ap()

    d_pdlin = nc.dram_tensor("pd_lin", [2 * E, V], bf16, kind="ExternalInput").ap()
    d_mdproj = nc.dram_tensor("md_proj", [V, DEC_H], bf16, kind="ExternalInput").ap()
    d_mdlin = nc.dram_tensor("md_lin", [DEC_H, MEL_DIMS], bf16, kind="ExternalInput").ap()
    d_mdlinb_row = nc.dram_tensor("md_lin_b_row", [1, MEL_DIMS], bf16, kind="ExternalInput").ap()
    d_mdprojb = nc.dram_tensor("md_proj_b", [DEC_H], f32, kind="ExternalInput").ap()
    d_pdlinb = nc.dram_tensor("pd_lin_b", [V], f32, kind="ExternalInput").ap()

    d_out = nc.dram_tensor("out", [ns, T, V + MEL_DIMS], f32, kind="ExternalOutput").ap()

    with tile.TileContext(nc) as tc:
        cpool = tc.alloc_tile_pool(name="consts", bufs=1)
        wring = tc.alloc_tile_pool(name="wring", bufs=2)
        apool = tc.alloc_tile_pool(name="acts", bufs=1)
        xring = tc.alloc_tile_pool(name="xmring", bufs=12)
        spool = tc.alloc_tile_pool(name="scratch", bufs=2)
        psA = tc.alloc_tile_pool(name="psumA", bufs=6, space="PSUM")
        psB = tc.alloc_tile_pool(name="psumB", bufs=2, space="PSUM")

        def ps_conv(name="ps"):
            # conv accumulators: private ring so sparse phases can't gate them
            return psA.tile([128, 512], f32, tag="psa", name=name)

        def ps_tile(name="ps"):
            return psB.tile([128, 512], f32, tag="psb", name=name)

        def ps_tile_bf(name="psb"):
            # PE transpose writes through in the input dtype
            return psB.tile([128, 512], bf16, tag="psb", name=name)

        # ---------------- constants / resident weights ----------------
        ones_row = cpool.tile([1, 512], bf16, name="ones_row")
        nc.vector.memset(ones_row, 1.0)
        ones_col = cpool.tile([128, 1], bf16, name="ones_col")
        nc.vector.memset(ones_col, 1.0)
        ident = cpool.tile([128, 128], bf16, name="ident")
        make_identity(nc, ident)

        meproj81_sb = cpool.tile([MEL_DIMS + 1, E], bf16, name="meproj81_sb")
        nc.sync.dma_start(meproj81_sb, d_meproj81)
        pdlin_sb = cpool.tile([128, HB, V], bf16, name="pdlin_sb")
        for c in range(HB):
            nc.sync.dma_start(pdlin_sb[:, c, :], d_pdlin[c * 128:(c + 1) * 128, :])
        mdproj_sb = cpool.tile([128, VB, DEC_H], bf16, name="mdproj_sb")
        for c in range(VB):
            nc.sync.dma_start(mdproj_sb[:, c, :], d_mdproj[c * 128:(c + 1) * 128, :])
        mdlin_sb = cpool.tile([128, MEL_DIMS], bf16, name="mdlin_sb")
        nc.sync.dma_start(mdlin_sb, d_mdlin)
        mdlinb_row = cpool.tile([1, MEL_DIMS], bf16, name="mdlinb_row")
        nc.sync.dma_start(mdlinb_row, d_mdlinb_row)

        # per-partition bias/scale tables [128, L, 4, nh]
        tabs = {}
        for g, L, Co in (('me', 4, E), ('pe', 4, E), ('pd', 2, 2 * E), ('md', 2, DEC_H)):
            t_ = cpool.tile([128, L, 4, Co // 128], f32, name=f"tab_{g}")
            nc.sync.dma_start(t_, d_tab[g])
            tabs[g] = t_
        b_mdproj = cpool.tile([128, 1], f32, name="b_mdproj")
        nc.sync.dma_start(b_mdproj, d_mdprojb.rearrange("(a p) -> p a", p=128))
        b_pdlin = cpool.tile([128, VB], f32, name="b_pdlin")
        nc.sync.dma_start(b_pdlin, d_pdlinb.rearrange("(a p) -> p a", p=128))

        # ---------------- persistent activations (per-sample reuse) ------
        ph2 = [apool.tile([128, 3 * 128], bf16, name=f"ph2_{c}") for c in range(EB)]
        phT = [apool.tile([128, E], bf16, name=f"phT{i}") for i in range(len(SBLK))]
        mdx = [apool.tile([128, TP], bf16, name="mdx0")]
        logitbf = [apool.tile([128, TP], bf16, name=f"logitbf{v}") for v in range(VB)]
        pen_sb = apool.tile([1, 3 * 128], bf16, name="pen_sb")

        def zero_borders(tiles):
            for t_ in tiles:
                w = t_.shape[1]
                nc.vector.memset(t_[:, 0:1], 0.0)
                nc.vector.memset(t_[:, w - 1:w], 0.0)

        zero_borders(mdx)

        # ---------------- conv helpers ----------------
        def pair_ap(t3, b0, b1, W):
            """Moving AP for a DR pass: blocks b0=(c0,k0), b1=(c1,k1) read from
            the packed fp8 tile t3 [rows, NC, XMW] (middle-dim stride >= 2)."""
            (c0, k0), (c1, k1) = b0, b1
            a0 = t3[:, c0, k0:k0 + W]
            d = t3[:, c1, k1:k1 + 1].offset - t3[:, c0, k0:k0 + 1].offset
            return bass.AP(tensor=a0.tensor, offset=a0.offset,
                           ap=[list(a0.ap[0]), [d, 2], [1, W]])

        def wpair_ap(wt, b0, b1, h):
            """Stationary AP for a DR pass from packed weights [rows, NB, Co]."""
            i0 = b0[0] * 3 + b0[1]
            i1 = b1[0] * 3 + b1[1]
            a0 = wt[:, i0, h * 128:(h + 1) * 128]
            d = wt[:, i1, 0:1].offset - wt[:, i0, 0:1].offset
            return bass.AP(tensor=a0.tensor, offset=a0.offset,
                           ap=[list(a0.ap[0]), [d, 2], [1, 128]])

        def conv_passes(n_cin):
            """DR pass plan over blocks (c,k): (c,k0)+(c,k2) pairs (stride 2),
            then k1 blocks paired across cin (stride XMW), odd k1 single."""
            p = [('dr', (c, 0), (c, 2)) for c in range(n_cin)]
            k1 = [(c, 1) for c in range(n_cin)]
            p += [('dr', k1[2 * i], k1[2 * i + 1]) for i in range(n_cin // 2)]
            if n_cin % 2:
                p.append(('single', k1[-1], None))
            return p

        def emit_half(ps, W, n_cin, fp8, wt, h, xm, xm8, i):
            if fp8:
                passes = conv_passes(n_cin)
                for j, (kind, b0, b1) in enumerate(passes):
                    if kind == 'dr':
                        nc.tensor.matmul(ps[:, :W], wpair_ap(wt, b0, b1, h),
                                         pair_ap(xm8[i], b0, b1, W),
                                         start=(j == 0), stop=(j == len(passes) - 1),
                                         perf_mode=DR)
                    else:
                        c, k = b0
                        nc.tensor.matmul(ps[:, :W],
                                         wt[:, c * 3 + k, h * 128:(h + 1) * 128],
                                         xm8[i][:, c, k:k + W],
                                         start=(j == 0), stop=(j == len(passes) - 1))
            else:
                nmm = 3 * n_cin
                idx = 0
                split = isinstance(wt, tuple)
                hb = n_cin
                for c in range(n_cin):
                    for k in range(3):
                        b = c * 3 + k
                        wsrc = (wt[b // hb][:, b % hb, h * 128:(h + 1) * 128]
                                if split else wt[:, b, h * 128:(h + 1) * 128])
                        nc.tensor.matmul(ps[:, :W], wsrc,
                                         xm[(c, i)][:, k:k + W],
                                         start=(idx == 0), stop=(idx == nmm - 1))
                        idx += 1

        def glu_layer(x_tiles, n_cin, tiles_list, TPAD, mask_rep, g, l,
                      wa_t, wg_t, pre_chunk_hook=None, pre_tile_hook=None):
            """One masked GLU conv block, in place on x_tiles (bf16).

            g: group name (QCONF + tabs key), l: layer for table lookup.
            wa_t/wg_t: packed SBUF weight tiles [128, 3*NC, Co].
            pre_chunk_hook(i)/pre_tile_hook(i): interleave points (attention,
            logits etc. ride under the dense conv matmuls).
            """
            a8, g8 = QCONF[g]['a'][l], QCONF[g]['g'][l]
            tab = tabs[g]
            n_half = n_cin  # cout == 2*cin for every GLU here
            xm = {}
            xm8 = {}

            def emit_xm(i):
                if pre_chunk_hook is not None:
                    pre_chunk_hook(i)
                t0, W = tiles_list[i]
                cw = min(W + 2, TPAD - t0)
                for c in range(n_cin):
                    xt = xring.tile([128, XMW], bf16, tag="xm", bufs=12,
                                    name=f"xm{c}_{i}")
                    nc.vector.tensor_tensor(
                        out=xt[:, :cw], in0=x_tiles[c][:, t0:t0 + cw],
                        in1=mask_rep[:, t0:t0 + cw], op=aop.mult)
                    xm[(c, i)] = xt
                if a8 or g8:
                    x8 = xring.tile([128, n_cin, XMW], f8e4, tag="xm8", bufs=2,
                                    name=f"xm8_{i}")
                    for c in range(n_cin):
                        nc.vector.tensor_tensor(
                            out=x8[:, c, :cw], in0=x_tiles[c][:, t0:t0 + cw],
                            in1=mask_rep[:, t0:t0 + cw], op=aop.mult)
                    xm8[i] = x8

            emit_xm(0)
            for i, (t0, W) in enumerate(tiles_list):
                if i + 1 < len(tiles_list):
                    emit_xm(i + 1)
                if pre_tile_hook is not None:
                    pre_tile_hook(i)
                for h in range(n_half):
                    a_ps = ps_conv("a_ps")
                    g_ps = ps_conv("g_ps")
                    emit_half(a_ps, W, n_cin, a8, wa_t, h, xm, xm8, i)
                    emit_half(g_ps, W, n_cin, g8, wg_t, h, xm, xm8, i)
                    sig = spool.tile([128, 512], bf16, tag="sig", bufs=2, name="sig")
                    nc.scalar.activation(sig[:, :W], g_ps[:, :W], AF.Sigmoid,
                                         bias=tab[:, l, 2, h:h + 1],
                                         scale=tab[:, l, 3, h:h + 1] if g8 else 1.0)
                    tmp = spool.tile([128, 512], bf16, tag="tmp", bufs=2, name="tmp")
                    nc.vector.scalar_tensor_tensor(
                        out=tmp[:, :W], in0=a_ps[:, :W],
                        scalar=tab[:, l, 0, h:h + 1],
                        in1=sig[:, :W], op0=aop.add, op1=aop.mult)
                    # x = tmp [* sa_inv] + xm   (masked residual)
                    if a8:
                        nc.vector.scalar_tensor_tensor(
                            out=x_tiles[h][:, 1 + t0:1 + t0 + W], in0=tmp[:, :W],
                            scalar=tab[:, l, 1, h:h + 1],
                            in1=xm[(h, i)][:, 1:1 + W], op0=aop.mult, op1=aop.add)
                    else:
                        nc.vector.tensor_tensor(
                            out=x_tiles[h][:, 1 + t0:1 + t0 + W], in0=tmp[:, :W],
                            in1=xm[(h, i)][:, 1:1 + W], op=aop.add)

        def _wring_bufs(R, NB, Co, dt8):
            """Ring depth per (shape, dtype) class: max tiles concurrently
            live in one layer (a+g share classes) + 1 for prefetch."""
            n_live = 0
            for g, (L, R_, NB_, Co_) in GSHAPE.items():
                for l in range(L):
                    if g == 'me' and l == 0:
                        continue
                    live = sum(1 for h in ('a', 'g')
                               if (R_, NB_, Co_) == (R, NB, Co)
                               and (wdt(g, h, l) == f8e4) == dt8)
                    # split bf16 halves count double
                    n_live = max(n_live, live)
            return n_live + 1

        def load_w(g, l):
            """Load one layer's packed conv weights (both halves).  Ring tags
            are shared by (shape, dtype) class so me/pe/pd rotate through the
            same SBUF buffers.  Large bf16 tiles (dec 'a') split in two along
            the block dim — fine for bf16 (no DR pairing constraints)."""
            out = []
            for h in ('a', 'g'):
                dtensor = d_w[(g, h, l)]
                R, NB, Co = dtensor.shape
                dt = wdt(g, h, l)
                if dt == bf16 and NB * Co * 2 > 16384:
                    hb = NB // 3
                    wts = []
                    for p_ in range(3):
                        wt = wring.tile([R, hb, Co], dt, tag=f"w{R}x{hb}x{Co}_b",
                                        bufs=4, name=f"w_{g}_{h}_{l}_{p_}")
                        nc.sync.dma_start(wt, dtensor[:, p_ * hb:(p_ + 1) * hb, :])
                        wts.append(wt)
                    out.append(tuple(wts))
                else:
                    dt8 = dt == f8e4
                    wt = wring.tile([R, NB, Co], dt,
                                    tag=f"w{R}x{NB}x{Co}_{'8' if dt8 else 'b'}",
                                    bufs=2 if g == 'me0'
                                    else _wring_bufs(R, NB, Co, dt8),
                                    name=f"w_{g}_{h}_{l}")
                    nc.sync.dma_start(wt, dtensor)
                    out.append(wt)
            return out

        def bcast_row(row_ap):
            # [1, N] AP -> stride-0 partition broadcast AP [128, N] for DMA
            return bass.AP(tensor=row_ap.tensor, offset=row_ap.offset,
                           ap=[[0, 128]] + [list(d) for d in row_ap.ap[1:]])

        def emit_ph_phase(s2):
            """Phoneme embedding + encoder + attention prep for sample s2.
            Emitted one sample ahead (after s2-1's decoder) so the PE never
            drains at sample boundaries."""
            phx = [apool.tile([128, SP], bf16, tag=f"phx{c}", bufs=1,
                              name=f"phx{c}") for c in range(EB)]
            zero_borders(phx)
            phmask = spool.tile([128, SP], bf16, tag="phmask", bufs=1,
                                name="phmask")
            nc.sync.dma_start(out=phmask, in_=bcast_row(d_phmask[s2:s2 + 1, :]))
            for c in range(EB):
                nc.sync.dma_start(phx[c], d_embph[s2, c * 128:(c + 1) * 128, :])
            for l in range(4):
                wa_t, wg_t = load_w('pe', l)
                glu_layer(phx, EB, [(0, S)], SP, phmask, 'pe', l, wa_t, wg_t)

            # ---- attention prep: ph2 = 2*ph_enc, p2 = sum(ph^2), phT ----
            p2_ps = ps_tile("p2_ps")
            for c in range(EB):
                nc.vector.tensor_scalar_mul(ph2[c][:, :S], phx[c][:, 1:1 + S], 2.0)
                nc.vector.memset(ph2[c][:, S:], 0.0)
                sq = spool.tile([128, S], bf16, tag="sq", bufs=1, name="sq")
                nc.vector.tensor_tensor(sq[:, :S], phx[c][:, 1:1 + S],
                                        phx[c][:, 1:1 + S], op=aop.mult)
                nc.tensor.matmul(p2_ps[0:1, :S], ones_col, sq[:, :S],
                                 start=(c == 0), stop=(c == EB - 1))
            phpen_f = spool.tile([1, S], f32, tag="phpen", bufs=1, name="phpen_f")
            nc.sync.dma_start(phpen_f, d_phpen[s2:s2 + 1, :])
            # pen = -p2 + phpen   (phpen is 0 valid / -1e9 masked)
            nc.vector.scalar_tensor_tensor(
                out=pen_sb[0:1, :S], in0=p2_ps[0:1, :S], scalar=-1.0,
                in1=phpen_f[0:1, :S], op0=aop.mult, op1=aop.add)
            nc.vector.memset(pen_sb[0:1, S:], -1e9)
            for c in range(EB):
                for si, (s0, sw) in enumerate(SBLK):
                    tr = ps_tile_bf("trph_ps")
                    nc.tensor.transpose(tr[:sw, :128],
                                        phx[c][:, 1 + s0:1 + s0 + sw],
                                        ident)
                    nc.vector.tensor_copy(phT[si][:sw, c * 128:(c + 1) * 128],
                                          tr[:sw, :128])

        # ================ per-sample pipeline ================
        for s in range(ns):
            # beyond mel_len the masked convs see zeros, so everything is a
            # per-channel constant: compute t < Teff only, broadcast the tail
            Teff = teffs[s]
            TTs = _ttiles(Teff)
            TBLKs = _ttiles(Teff, 128)
            if s == 0:
                emit_ph_phase(0)
            decx = [apool.tile([128, TP], bf16, tag=f"decx{c}",
                               bufs=2 if c < EB else 1, name=f"decx{c}")
                    for c in range(HB)]
            zero_borders(decx)
            if s < 2 and Teff < T:
                for t_ in decx + mdx + logitbf:
                    nc.vector.memset(t_[:, 1 + Teff:TP - 1], 0.0)
            # ---- mel mask (partition-broadcast via stride-0 DMA) ----
            melmask = spool.tile([128, TP], bf16, tag="melmask", bufs=1,
                                 name="melmask")
            nc.sync.dma_start(out=melmask, in_=bcast_row(d_melmask[s:s + 1, :]))

            # ---- masked mels tape [mels*mask; mask] for the fused proj/L0;
            # row 80 of the dram mels is ones, so one mask-multiply fills
            # both the masked mels and the mask row ----
            mels_m = spool.tile([MEL_DIMS + 1, TP], bf16, tag="melsm", bufs=1,
                                name="mels_m")
            nc.sync.dma_start(mels_m[:, 1:1 + Teff], d_mels[s, :, :Teff])
            nc.vector.memset(mels_m[:, 0:1], 0.0)
            nc.vector.memset(mels_m[:, TP - 1:TP], 0.0)

            def mel0_layer():
                """Fused mel projection + first GLU layer: conv contraction is
                the 81-row masked-mels tape (w_me0 = P81 @ me_W[0]); the proj
                output (= masked residual) comes from the same tape."""
                wa0, wg0 = load_w('me0', 0)
                a8, g8 = QCONF['me0']['a'][0], QCONF['me0']['g'][0]
                tab = tabs['me']
                x0 = {}
                t8 = {}

                def emit_tape8(i):
                    # cast tile i's tape to fp8 -- callable only once tile
                    # i+1's mask-mult has filled the right halo columns
                    t0, W = TTs[i]
                    cw = min(W + 2, TP - t0)
                    x8 = xring.tile([MEL_DIMS + 1, 1, XMW], f8e4, tag="tape8",
                                    bufs=3, name=f"tape8_{i}")
                    nc.vector.tensor_copy(x8[:, 0, :cw],
                                          mels_m[0:MEL_DIMS + 1, t0:t0 + cw])
                    t8[i] = x8

                def emit_proj(i):
                    t0, W = TTs[i]
                    nc.vector.tensor_tensor(
                        out=mels_m[0:MEL_DIMS + 1, 1 + t0:1 + t0 + W],
                        in0=mels_m[0:MEL_DIMS + 1, 1 + t0:1 + t0 + W],
                        in1=melmask[0:MEL_DIMS + 1, 1 + t0:1 + t0 + W],
                        op=aop.mult)
                    if i == len(TTs) - 1 and Teff < T:
                        nc.vector.memset(
                            mels_m[:, 1 + Teff:min(TP, 3 + Teff)], 0.0)
                    if a8 or g8:
                        if i > 0:
                            emit_tape8(i - 1)
                        if i == len(TTs) - 1:
                            emit_tape8(i)
                    for eb in range(EB):
                        ps = ps_conv("proj_ps")
                        nc.tensor.matmul(ps[:, :W],
                                         meproj81_sb[:, eb * 128:(eb + 1) * 128],
                                         mels_m[0:MEL_DIMS + 1, 1 + t0:1 + t0 + W],
                                         start=True, stop=True)
                        xt = spool.tile([128, 512], bf16, tag="x0", bufs=6,
                                        name=f"x0_{eb}")
                        nc.vector.tensor_copy(xt[:, :W], ps[:, :W])
                        x0[(eb, i)] = xt

                def emit_half0(ps, W, i, t0, fp8, wt, h):
                    if fp8:
                        nc.tensor.matmul(ps[:, :W], wpair_ap(wt, (0, 0), (0, 2), h),
                                         pair_ap(t8[i], (0, 0), (0, 2), W),
                                         start=True, stop=False, perf_mode=DR)
                        nc.tensor.matmul(ps[:, :W],
                                         wt[:, 1, h * 128:(h + 1) * 128],
                                         t8[i][:, 0, 1:1 + W],
                                         start=False, stop=True)
                    else:
                        for k in range(3):
                            nc.tensor.matmul(
                                ps[:, :W], wt[:, k, h * 128:(h + 1) * 128],
                                mels_m[0:MEL_DIMS + 1, t0 + k:t0 + k + W],
                                start=(k == 0), stop=(k == 2))

                emit_proj(0)
                for i, (t0, W) in enumerate(TTs):
                    if i + 1 < len(TTs):
                        emit_proj(i + 1)
                    for h in range(EB):
                        a_ps = ps_conv("a_ps")
                        g_ps = ps_conv("g_ps")
                        emit_half0(a_ps, W, i, t0, a8, wa0, h)
                        emit_half0(g_ps, W, i, t0, g8, wg0, h)
                        sig = spool.tile([128, 512], bf16, tag="sig", bufs=2,
                                         name="sig")
                        nc.scalar.activation(sig[:, :W], g_ps[:, :W], AF.Sigmoid,
                                             bias=tab[:, 0, 2, h:h + 1],
                                             scale=tab[:, 0, 3, h:h + 1] if g8 else 1.0)
                        tmp = spool.tile([128, 512], bf16, tag="tmp", bufs=2,
                                         name="tmp")
                        nc.vector.scalar_tensor_tensor(
                            out=tmp[:, :W], in0=a_ps[:, :W],
                            scalar=tab[:, 0, 0, h:h + 1],
                            in1=sig[:, :W], op0=aop.add, op1=aop.mult)
                        if a8:
                            nc.vector.scalar_tensor_tensor(
                                out=decx[h][:, 1 + t0:1 + t0 + W], in0=tmp[:, :W],
                                scalar=tab[:, 0, 1, h:h + 1],
                                in1=x0[(h, i)][:, :W], op0=aop.mult, op1=aop.add)
                        else:
                            nc.vector.tensor_tensor(
                                out=decx[h][:, 1 + t0:1 + t0 + W], in0=tmp[:, :W],
                                in1=x0[(h, i)][:, :W], op=aop.add)

            def attn_group(gi, s=s, decx=decx):
                g0, GW = TTs[gi]
                atg = [spool.tile([128, 512], bf16, tag=f"attnT{si}", bufs=1,
                                  name=f"attnT{si}") for si in range(len(SBLK))]

                def transposes(pend):
                    # deferred one block so the PE rides the next block's
                    # scores while this block's softmax finishes
                    TWp, jp, attnp = pend
                    for si, (s0, sw) in enumerate(SBLK):
                        tr = ps_tile_bf("trat_ps")
                        nc.tensor.transpose(tr[:sw, :TWp],
                                            attnp[:TWp, s0:s0 + sw],
                                            ident[:TWp, :TWp])
                        nc.vector.tensor_copy(atg[si][:sw, jp * 128:jp * 128 + TWp],
                                              tr[:sw, :TWp])

                pend = None
                for t0 in range(g0, g0 + GW, 128):
                    TW = min(128, g0 + GW - t0)
                    j = (t0 - g0) // 128
                    s_ps = ps_tile("s_ps")
                    for c in range(EB):
                        nc.tensor.matmul(s_ps[:TW, :S],
                                         decx[c][:, 1 + t0:1 + t0 + TW],
                                         ph2[c][:, :S], start=(c == 0), stop=False)
                    nc.tensor.matmul(s_ps[:TW, :S], ones_row[:, :TW],
                                     pen_sb[0:1, :S], start=False, stop=True)
                    negmx = spool.tile([128, 1], f32, tag="negmx", bufs=4, name="negmx")
                    nc.vector.reduce_max(negmx[:TW], s_ps[:TW, :S], axis=AX.X,
                                         negate=True)
                    attn = spool.tile([128, S], bf16, tag="attn", bufs=2, name="attn")
                    sumexp = spool.tile([128, 1], f32, tag="sumexp", bufs=4,
                                        name="sumexp")
                    nc.scalar.activation(attn[:TW, :S], s_ps[:TW, :S], AF.Exp,
                                         bias=negmx[:TW], scale=1.0,
                                         accum_out=sumexp[:TW])
                    rcp = spool.tile([128, 1], f32, tag="rcp", bufs=4, name="rcp")
                    nc.vector.reciprocal(rcp[:TW], sumexp[:TW])
                    nc.vector.tensor_scalar_mul(attn[:TW, :S], attn[:TW, :S],
                                                rcp[:TW])
                    if pend is not None:
                        transposes(pend)
                    pend = (TW, j, attn)
                transposes(pend)
                for eb in range(EB):
                    ctx = ps_tile("ctx_ps")
                    for si, (s0, sw) in enumerate(SBLK):
                        nc.tensor.matmul(ctx[:, :GW],
                                         phT[si][:sw, eb * 128:(eb + 1) * 128],
                                         atg[si][:sw, :GW],
                                         start=(si == 0), stop=(si == len(SBLK) - 1))
                    nc.vector.tensor_copy(decx[EB + eb][:, 1 + g0:1 + g0 + GW],
                                          ctx[:, :GW])

            # ---- mel encoder: 4 GLU layers on decx[0:3]; attention group g
            # is emitted under layer 3's dense convs right after the tile
            # that produces its mel_enc columns ----
            def mel3_hook(i):
                if i >= 1:
                    attn_group(i - 1)

            mel0_layer()
            for l in range(1, 4):
                wa_t, wg_t = load_w('me', l)
                glu_layer(decx[:EB], EB, TTs, TP, melmask, 'me', l, wa_t, wg_t,
                          pre_tile_hook=mel3_hook if l == 3 else None)

            def dec0_chunk_hook(i):
                # the last attention group rides under dec L0's dense convs.
                # It must precede dec L0's xm for tile last-1, whose right
                # halo column reads the first ctx column of the last tile.
                if i == max(0, len(TTs) - 2):
                    attn_group(len(TTs) - 1)

            # ---- logits (feature-major) -> logitbf, then mel_h0 ----
            def lgprep(ti):
                t0, W = TTs[ti]
                for vb in range(VB):
                    lf = ps_tile("lf_ps")
                    for c in range(HB):
                        nc.tensor.matmul(lf[:, :W],
                                         pdlin_sb[:, c, vb * 128:(vb + 1) * 128],
                                         decx[c][:, 1 + t0:1 + t0 + W],
                                         start=(c == 0), stop=(c == HB - 1))
                    nc.vector.tensor_scalar_add(logitbf[vb][:, 1 + t0:1 + t0 + W],
                                                lf[:, :W], b_pdlin[:, vb:vb + 1])
                mh = ps_tile("mh_ps")
                for vb in range(VB):
                    nc.tensor.matmul(mh[:, :W], mdproj_sb[:, vb, :],
                                     logitbf[vb][:, 1 + t0:1 + t0 + W],
                                     start=(vb == 0), stop=(vb == VB - 1))
                nc.vector.tensor_scalar_add(mdx[0][:, 1 + t0:1 + t0 + W],
                                            mh[:, :W], b_mdproj[:, 0:1])

            # ---- log_softmax tiles (emitted interleaved with conv work).
            # One Ln per 512-tile (4 blocks) keeps the scalar engine from
            # thrashing activation tables between Exp and Ln. ----
            nblk = len(TBLKs)
            outlps = []

            def lg_tile(ti, s=s, outlps=outlps):
                blocks = list(blocks_of(ti))
                se2a = spool.tile([128, 4], f32, tag="se2a", bufs=2, name="se2a")
                pend = []
                for j, tb in enumerate(blocks):
                    t0, TW = TBLKs[tb]
                    lg = ps_tile_bf("lg_ps")
                    for vb in range(VB):
                        nc.tensor.transpose(lg[:TW, vb * 128:(vb + 1) * 128],
                                            logitbf[vb][:, 1 + t0:1 + t0 + TW],
                                            ident)
                    negmx2 = spool.tile([128, 1], f32, tag="negmx2", bufs=5,
                                        name="negmx2")
                    nc.vector.reduce_max(negmx2[:TW], lg[:TW, :V], axis=AX.X,
                                         negate=True)
                    esc = spool.tile([128, V], bf16, tag="esc", bufs=1, name="esc")
                    nc.scalar.activation(esc[:TW, :V], lg[:TW, :V], AF.Exp,
                                         bias=negmx2[:TW], scale=1.0,
                                         accum_out=se2a[:TW, j:j + 1])
                    outlp = spool.tile([128, V], f32, tag="outlp", bufs=5,
                                       name="outlp")
                    nc.vector.tensor_scalar_add(outlp[:TW, :V], lg[:TW, :V],
                                                negmx2[:TW])
                    pend.append((tb, t0, TW, outlp))
                lna = spool.tile([128, 4], f32, tag="lna", bufs=2, name="lna")
                nc.scalar.activation(lna[:, :len(blocks)], se2a[:, :len(blocks)],
                                     AF.Ln)
                for j, (tb, t0, TW, outlp) in enumerate(pend):
                    nc.vector.tensor_scalar_sub(outlp[:TW, :V], outlp[:TW, :V],
                                                lna[:TW, j:j + 1])
                    nc.sync.dma_start(d_out[s, t0:t0 + TW, 0:V], outlp[:TW, :V])
                    outlps.append(outlp)

            def preds_block(tb, s=s, mdx=mdx):
                t0, TW = TBLKs[tb]
                mp = ps_tile("mp_ps")
                nc.tensor.matmul(mp[:TW, :MEL_DIMS], mdx[0][:, 1 + t0:1 + t0 + TW],
                                 mdlin_sb[:, :MEL_DIMS], start=True, stop=False)
                nc.tensor.matmul(mp[:TW, :MEL_DIMS], ones_row[:, :TW], mdlinb_row,
                                 start=False, stop=True)
                outmp = spool.tile([128, MEL_DIMS], f32, tag="outmp", bufs=2,
                                   name="outmp")
                s2 = spool.tile([128, MEL_DIMS], f32, tag="s2", bufs=2, name="s2")
                nc.scalar.activation(s2[:TW, :MEL_DIMS], mp[:TW, :MEL_DIMS],
                                     AF.Sigmoid, scale=2.0)
                nc.vector.tensor_scalar(out=outmp[:TW, :MEL_DIMS],
                                        in0=s2[:TW, :MEL_DIMS], scalar1=2.0,
                                        scalar2=-1.0, op0=aop.mult, op1=aop.add)
                nc.sync.dma_start(d_out[s, t0:t0 + TW, V:V + MEL_DIMS],
                                  outmp[:TW, :MEL_DIMS])
                outmps[tb] = outmp

            outmps = {}
            blocks_of = lambda i: range(4 * i, min(4 * i + 4, nblk))

            def dec1_hook(i):
                # tile i-1 of dec output is final: emit its logits work under
                # this tile's dense conv matmuls
                if i >= 1:
                    lgprep(i - 1)
                    lg_tile(i - 1)

            # ---- decoder: 2 GLU layers on decx[0:6] ----
            for l in range(2):
                wa_t, wg_t = load_w('pd', l)
                glu_layer(decx, HB, TTs, TP, melmask, 'pd', l, wa_t, wg_t,
                          pre_tile_hook=dec1_hook if l == 1 else None,
                          pre_chunk_hook=dec0_chunk_hook if l == 0 else None)
            lgprep(len(TTs) - 1)
            lg_tile(len(TTs) - 1)

            # ---- next sample's phoneme phase rides here: its PE work fills
            # the mel-decoder lull and the sample boundary ----
            if s + 1 < ns:
                emit_ph_phase(s + 1)

            def md_hook1(i):
                if i > 0:
                    for tb in blocks_of(i - 1):
                        preds_block(tb)

            # ---- mel decoder: 2 GLU layers on mdx, sparse phases woven in ----
            for l in range(2):
                wa_t, wg_t = load_w('md', l)
                glu_layer(mdx, 1, TTs, TP, melmask, 'md', l, wa_t, wg_t,
                          pre_tile_hook=md_hook1 if l == 1 else None)
            for tb in blocks_of(len(TTs) - 1):
                preds_block(tb)

            if Teff < T:
                # broadcast the constant row Teff-1 over the skipped tail:
                # replicate it across partitions, then tile DMAs of <=128 rows
                row = TBLKs[-1][1] - 1
                rowcat = spool.tile([128, V + MEL_DIMS], f32, tag="rowcat",
                                    bufs=1, name="rowcat")
                nc.sync.dma_start(rowcat[0:1, 0:V], outlps[-1][row:row + 1, :V])
                nc.sync.dma_start(rowcat[0:1, V:V + MEL_DIMS],
                                  outmps[len(TBLKs) - 1][row:row + 1, :MEL_DIMS])
                nc.gpsimd.partition_broadcast(rowcat[:, :], rowcat[0:1, :])
                for r0 in range(Teff, T, 128):
                    rw = min(128, T - r0)
                    nc.sync.dma_start(d_out[s, r0:r0 + rw, :], rowcat[:rw, :])

        psB.release()
        psA.release()
        spool.release()
        xring.release()
        apool.release()
        wring.release()
        cpool.release()

    nc.compile()
    return nc


def plan_slots(mel_lens, ns=NS, T=T_MEL, n_cores=N_CORES, margin=12):
    """Sort samples by length; slot j of every core gets rank 8j+core.
    Returns (order, teffs): order[core*ns + slot] = original sample index,
    teffs[slot] = compile-time effective length for that slot (same on all
    cores, so a single SPMD NEFF serves all 8)."""
    mel_lens = np.asarray(mel_lens).astype(np.int64)
    idx = np.argsort(-mel_lens, kind='stable')
    order = np.empty(ns * n_cores, np.int64)
    teffs = []
    for j in range(ns):
        grp = idx[j * n_cores:(j + 1) * n_cores]
        for c in range(n_cores):
            order[c * ns + j] = grp[c]
        te = int(mel_lens[grp].max()) + margin
        te = min(T, ((te + 127) // 128) * 128)
        teffs.append(te)
    return order, tuple(teffs)


def _pack_conv(w, q):
    """w: [3, C, Co] f32 one-layer master -> packed [R, 3*NC, Co] (+ scale).

    Block b = c*3 + k.  q=True: per-cout-channel scale s_j = 224/absmax,
    values clipped to +-240 and stored fp8; else bf16."""
    K, C, Co = w.shape
    if C % 128 == 0:
        R, NC = 128, C // 128
    else:
        R, NC = C, 1
    if q:
        am = np.abs(w).reshape(-1, Co).max(0)
        s = (224.0 / np.maximum(am, 1e-9)).astype(np.float32)
        wq = np.clip(w * s[None, None, :], -240.0, 240.0)
    else:
        s, wq = None, w
    arr = wq.reshape(3, NC, R, Co).transpose(2, 1, 0, 3)
    arr = np.ascontiguousarray(arr.reshape(R, 3 * NC, Co))
    return arr.astype(F8 if q else BF), s


def _tab_row(b, Co, sa, sg):
    """One layer's bias/scale table [4, Co]: (b_a_eff, sa_inv, b_g, sg_inv)."""
    t = np.zeros((4, Co), np.float32)
    ba, bg = b[:Co], b[Co:]
    t[0] = ba * (sa if sa is not None else 1.0)
    t[1] = (1.0 / sa) if sa is not None else 1.0
    t[2] = bg
    t[3] = (1.0 / sg) if sg is not None else 1.0
    return t


def preprocess(inputs, ns=NS, T=T_MEL, TPH=T_PHON, n_cores=N_CORES, order=None):
    """Host-side prep: transpose/pad/cast, build masks, pack weights, shard."""
    S = TPH + 1
    TP = T + 2
    SP = S + 2
    B = ns * n_cores

    mels = np.asarray(inputs['mels'], np.float32)[:B, :T]
    phonemes = np.asarray(inputs['phonemes']).astype(np.int64)[:B, :TPH]
    mel_lens = np.asarray(inputs['mel_lens']).astype(np.int64)[:B]
    phoneme_lens = np.asarray(inputs['phoneme_lens']).astype(np.int64)[:B]
    if order is not None:
        mels = mels[order]
        phonemes = phonemes[order]
        mel_lens = mel_lens[order]
        phoneme_lens = phoneme_lens[order]
    emb = np.asarray(inputs['emb'], np.float32)

    mels_t = np.concatenate(
        [mels.transpose(0, 2, 1), np.ones((B, 1, T), np.float32)],
        axis=1).astype(BF)  # [B, 81, T]; row 80 = ones (mask source)

    ph = np.concatenate([np.zeros((B, 1), np.int64), phonemes], axis=1)  # [B,S]
    embph = emb[ph]                                    # [B, S, E] f32
    embph_t = np.zeros((B, E, SP), np.float32)
    embph_t[:, :, 1:1 + S] = embph.transpose(0, 2, 1)
    embph_t = embph_t.astype(BF)

    t_idx = np.arange(T)
    melmask = np.zeros((B, TP), np.float32)
    melmask[:, 1:1 + T] = (t_idx[None, :] < mel_lens[:, None]).astype(np.float32)
    melmask = melmask.astype(BF)

    s_idx = np.arange(S)
    ph_valid = s_idx[None, :] <= phoneme_lens[:, None]
    phmask = np.zeros((B, SP), np.float32)
    phmask[:, 1:1 + S] = ph_valid.astype(np.float32)
    phmask = phmask.astype(BF)
    phpen = np.where(ph_valid, 0.0, -1e9).astype(np.float32)  # [B, S]

    # fold the mel projection into the first mel GLU layer: with the mask
    # applied to raw mels, conv(proj(mels)*mask) == (mels*mask) @ (P @ W0_k),
    # and the proj bias contributes exactly b@W0_k per position times the
    # mask value -- append the mask itself as input row 80 (P' row 80 = b).
    P81 = np.concatenate([np.asarray(inputs['me_proj_W'], np.float64),
                          np.asarray(inputs['me_proj_b'], np.float64)[None]], 0)
    me_w0 = np.einsum('ce,kef->kcf', P81,
                      np.asarray(inputs['me_W'], np.float64)[0]).astype(np.float32)

    shared = {
        'me_proj81': P81.astype(np.float32).astype(BF),
        'pd_lin': np.asarray(inputs['pd_lin_W'], np.float32).astype(BF),
        'md_proj': np.asarray(inputs['md_proj_W'], np.float32).astype(BF),
        'md_lin': np.asarray(inputs['md_lin_W'], np.float32).astype(BF),
        'md_lin_b_row': np.asarray(inputs['md_lin_b'], np.float32)[None, :].astype(BF),
        'md_proj_b': np.asarray(inputs['md_proj_b'], np.float32),
        'pd_lin_b': np.asarray(inputs['pd_lin_b'], np.float32),
    }

    # packed conv weights + tables
    masters = {
        'me0': (me_w0[None], None),
        'me': (np.asarray(inputs['me_W'], np.float32), 'me_b'),
        'pe': (np.asarray(inputs['pe_W'], np.float32), 'pe_b'),
        'pd': (np.asarray(inputs['pd_W'], np.float32), 'pd_b'),
        'md': (np.asarray(inputs['md_W'], np.float32), 'md_b'),
    }
    scales = {}
    for g, (w, _) in masters.items():
        Co = w.shape[-1] // 2
        for h, sl in (('a', slice(0, Co)), ('g', slice(Co, None))):
            for l in range(w.shape[0]):
                if g == 'me' and l == 0:
                    continue
                arr, s = _pack_conv(np.ascontiguousarray(w[l, ..., sl]),
                                    QCONF[g][h][l])
                shared[f'w_{g}_{h}_{l}'] = arr
                scales[(g, h, l)] = s
    for g in ('me', 'pe', 'pd', 'md'):
        b = np.asarray(inputs[masters[g][1]], np.float32)
        L = b.shape[0]
        Co = b.shape[-1] // 2
        t = np.zeros((L, 4, Co), np.float32)
        for l in range(L):
            if g == 'me' and l == 0:
                # layer 0 of the mel encoder is the fused me0 conv
                t[0] = _tab_row(b[0], Co, scales[('me0', 'a', 0)],
                                scales[('me0', 'g', 0)])
            else:
                t[l] = _tab_row(b[l], Co, scales[(g, 'a', l)], scales[(g, 'g', l)])
        nh = Co // 128
        tp = t.reshape(L, 4, nh, 128).transpose(3, 0, 1, 2)
        shared[f'tab_{g}'] = np.ascontiguousarray(tp)

    in_maps = []
    for core in range(n_cores):
        sl = slice(core * ns, (core + 1) * ns)
        m = dict(shared)
        m['mels'] = np.ascontiguousarray(mels_t[sl])
        m['embph'] = np.ascontiguousarray(embph_t[sl])
        m['melmask'] = np.ascontiguousarray(melmask[sl])
        m['phmask'] = np.ascontiguousarray(phmask[sl])
        m['phpen'] = np.ascontiguousarray(phpen[sl])
        in_maps.append(m)
    return in_maps


_CACHE = {}


def _get_nc(teffs=None):
    key = teffs if teffs is not None else ('full',)
    if key not in _CACHE:
        _CACHE[key] = build(teffs=list(teffs) if teffs is not None else None)
    return _CACHE[key]


def kernel(**inputs) -> np.ndarray:
    from concourse.bass_utils import run_bass_kernel_spmd
    order, teffs = plan_slots(np.asarray(inputs['mel_lens']))
    nc = _get_nc(teffs)
    in_maps = preprocess(inputs, order=order)
    res = run_bass_kernel_spmd(nc, in_maps, core_ids=list(range(N_CORES)))
    out = np.concatenate([r['out'] for r in res.results], axis=0)
    inv = np.empty_like(order)
    inv[order] = np.arange(len(order))
    out = out[inv]
    return np.ascontiguousarray(out.astype(np.float32))


if __name__ == '__main__':
    import reference
    inputs = {k: np.asarray(v) for k, v in reference.setup_inputs().items()}
    out = kernel(**inputs)
    print(out.shape, out.dtype)


# revision 29
# speedup vs baseline: 1.0248x; 1.0248x over previous
"""Bass/Tile kernel for nn_AligningModel on 8 Trainium2 NeuronCores.

Data parallel: 32 samples sharded 4-per-core across 8 cores; all params
replicated.  Inside each core the model runs feature-major ([C, T] layout)
with bf16/fp8 matmul operands and fp32 PSUM accumulation:

  mel GLU encoder (4 layers)  -> mel_enc   [384, T]
  phoneme GLU encoder (4)     -> ph_enc    [384, 257]
  -L2^2 attention softmax     -> context   [384, T]   (the |mel|^2 term
      cancels inside softmax, so scores are 2*mel.ph - |ph|^2 - penalty)
  dec GLU (2 layers, 768ch)   -> dec       [768, T]
  logits twice (token-major for log_softmax, feature-major for mel decoder)
  mel GLU decoder (2 layers, 128ch) -> tanh mel preds

Convolutions (k=3, SAME) are shifted matmuls accumulated in PSUM.  Conv
halves selected in QCONF run in fp8-e4m3 DoubleRow: weights are packed
[rows, 3*NC, Co] so block pairs ((c,k0),(c,k2)) and ((c0,k1),(c1,k1)) form
DR passes whose moving APs read straight out of one packed fp8 activation
tile (middle-dim strides 2 and XMW — no interleave copies).  Per-cout
weight scales fold into the sigmoid scale / bias / residual multiplier.
"""

import numpy as np
import ml_dtypes

BF = ml_dtypes.bfloat16
F8 = ml_dtypes.float8_e4m3   # TRN fp8_e4m3 (max normal 240)

B_FULL = 32
T_MEL = 2000
T_PHON = 256
E = 384
V = 256
MEL_DIMS = 80
DEC_H = 128
N_CORES = 8
NS = B_FULL // N_CORES  # samples per core

# which conv halves run fp8-DoubleRow, per group and layer (mel L0 fused,
# mel enc L1-3 [slot 0 unused], phoneme enc, decoder, mel decoder)
QCONF = {
    'me0': {'a': [False], 'g': [False]},
    'me': {'a': [False] * 4, 'g': [False, True, True, False]},
    'pe': {'a': [False] * 4, 'g': [True] * 4},
    'pd': {'a': [False] * 2, 'g': [True] * 2},
    'md': {'a': [True] * 2, 'g': [True] * 2},
}


def _ttiles(T, w=512):
    out, t0 = [], 0
    while t0 < T:
        out.append((t0, min(w, T - t0)))
        t0 += w
    return out


def build(ns=NS, T=T_MEL, TPH=T_PHON, teffs=None):
    """Build and compile the per-core Bass kernel (ns samples, seq len T)."""
    import concourse.bacc as bacc
    import concourse.tile as tile
    import concourse.mybir as mybir
    from concourse.alu_op_type import AluOpType as aop
    from concourse.masks import make_identity
    import concourse.bass as bass

    f32 = mybir.dt.float32
    bf16 = mybir.dt.bfloat16
    f8e4 = mybir.dt.float8e4
    DR = mybir.MatmulPerfMode.DoubleRow
    AF = mybir.ActivationFunctionType
    AX = mybir.AxisListType

    if teffs is None:
        teffs = [T] * ns
    assert len(teffs) == ns and all(128 <= t <= T and t % 128 == 0 or t == T
                                    for t in teffs)
    S = TPH + 1          # phoneme positions incl. prepended blank
    TP = T + 2           # zero border column each side
    SP = S + 2
    TT = _ttiles(T)          # 512-wide t tiles
    TBLK = _ttiles(T, 128)   # 128-wide t blocks
    SBLK = _ttiles(S, 128)   # s chunks
    EB = E // 128            # 3
    HB = 2 * E // 128        # 6
    VB = V // 128            # 2
    XMW = 516                # xm chunk slot width (512 + 2 halo + margin)

    nc = bacc.Bacc("TRN2", debug=False, target_bir_lowering=False)

    def wdt(g, h, l):
        return f8e4 if QCONF[g][h][l] else bf16

    # ---------------- DRAM I/O ----------------
    d_mels = nc.dram_tensor("mels", [ns, MEL_DIMS + 1, T], bf16, kind="ExternalInput").ap()
    d_embph = nc.dram_tensor("embph", [ns, E, SP], bf16, kind="ExternalInput").ap()
    d_melmask = nc.dram_tensor("melmask", [ns, TP], bf16, kind="ExternalInput").ap()
    d_phmask = nc.dram_tensor("phmask", [ns, SP], bf16, kind="ExternalInput").ap()
    d_phpen = nc.dram_tensor("phpen", [ns, S], f32, kind="ExternalInput").ap()

    d_meproj81 = nc.dram_tensor("me_proj81", [MEL_DIMS + 1, E], bf16, kind="ExternalInput").ap()
    # packed conv weights, one tensor per layer: [rows, 3*NC, Co], b = c*3 + k
    GSHAPE = {'me0': (1, MEL_DIMS + 1, 3, E), 'me': (4, 128, 9, E),
              'pe': (4, 128, 9, E), 'pd': (2, 128, 18, 2 * E),
              'md': (2, 128, 3, DEC_H)}
    d_w = {}
    for g, (L, R, NB, Co) in GSHAPE.items():
        for h in ('a', 'g'):
            for l in range(L):
                if g == 'me' and l == 0:
                    continue
                d_w[(g, h, l)] = nc.dram_tensor(
                    f"w_{g}_{h}_{l}", [R, NB, Co], wdt(g, h, l),
                    kind="ExternalInput").ap()
    # bias/scale tables, host-packed partition-major: [128, L, 4, Co//128]
    d_tab = {}
    for g, L, Co in (('me', 4, E), ('pe', 4, E), ('pd', 2, 2 * E), ('md', 2, DEC_H)):
        d_tab[g] = nc.dram_tensor(f"tab_{g}", [128, L, 4, Co // 128], f32,
                                  kind="ExternalInput").ap()

    d_pdlin = nc.dram_tensor("pd_lin", [2 * E, V], bf16, kind="ExternalInput").ap()
    d_mdproj = nc.dram_tensor("md_proj", [V, DEC_H], bf16, kind="ExternalInput").ap()
    d_mdlin = nc.dram_tensor("md_lin", [DEC_H, MEL_DIMS], bf16, kind="ExternalInput").ap()
    d_mdlinb_row = nc.dram_tensor("md_lin_b_row", [1, MEL_DIMS], bf16, kind="ExternalInput").ap()
    d_mdprojb = nc.dram_tensor("md_proj_b", [DEC_H], f32, kind="ExternalInput").ap()
    d_pdlinb = nc.dram_tensor("pd_lin_b", [V], f32, kind="ExternalInput").ap()

    d_out = nc.dram_tensor("out", [ns, T, V + MEL_DIMS], f32, kind="ExternalOutput").ap()

    with tile.TileContext(nc) as tc:
        cpool = tc.alloc_tile_pool(name="consts", bufs=1)
        wring = tc.alloc_tile_pool(name="wring", bufs=2)
        apool = tc.alloc_tile_pool(name="acts", bufs=1)
        xring = tc.alloc_tile_pool(name="xmring", bufs=12)
        spool = tc.alloc_tile_pool(name="scratch", bufs=2)
        psA = tc.alloc_tile_pool(name="psumA", bufs=6, space="PSUM")
        psB = tc.alloc_tile_pool(name="psumB", bufs=2, space="PSUM")

        def ps_conv(name="ps"):
            # conv accumulators: private ring so sparse phases can't gate them
            return psA.tile([128, 512], f32, tag="psa", name=name)

        def ps_tile(name="ps"):
            return psB.tile([128, 512], f32, tag="psb", name=name)

        def ps_tile_bf(name="psb"):
            # PE transpose writes through in the input dtype
            return psB.tile([128, 512], bf16, tag="psb", name=name)

        # ---------------- constants / resident weights ----------------
        ones_row = cpool.tile([1, 512], bf16, name="ones_row")
        nc.vector.memset(ones_row, 1.0)
        ones_col = cpool.tile([128, 1], bf16, name="ones_col")
        nc.vector.memset(ones_col, 1.0)
        ident = cpool.tile([128, 128], bf16, name="ident")
        make_identity(nc, ident)

        meproj81_sb = cpool.tile([MEL_DIMS + 1, E], bf16, name="meproj81_sb")
        pdlin_sb = cpool.tile([128, HB, V], bf16, name="pdlin_sb")
        mdproj_sb = cpool.tile([128, VB, DEC_H], bf16, name="mdproj_sb")
        mdlin_sb = cpool.tile([128, MEL_DIMS], bf16, name="mdlin_sb")
        mdlinb_row = cpool.tile([1, MEL_DIMS], bf16, name="mdlinb_row")
        tabs = {}
        for g, L, Co in (('me', 4, E), ('pe', 4, E), ('pd', 2, 2 * E), ('md', 2, DEC_H)):
            tabs[g] = cpool.tile([128, L, 4, Co // 128], f32, name=f"tab_{g}")
        b_mdproj = cpool.tile([128, 1], f32, name="b_mdproj")
        b_pdlin = cpool.tile([128, VB], f32, name="b_pdlin")

        def load_consts_early():
            # needed by the first phoneme layers / mel0
            for g in ('pe', 'me'):
                nc.sync.dma_start(tabs[g], d_tab[g])
            nc.sync.dma_start(meproj81_sb, d_meproj81)

        def load_consts_late():
            # not needed until the decoder/logits of sample 0
            for g in ('pd', 'md'):
                nc.sync.dma_start(tabs[g], d_tab[g])
            for c in range(HB):
                nc.sync.dma_start(pdlin_sb[:, c, :],
                                  d_pdlin[c * 128:(c + 1) * 128, :])
            for c in range(VB):
                nc.sync.dma_start(mdproj_sb[:, c, :],
                                  d_mdproj[c * 128:(c + 1) * 128, :])
            nc.sync.dma_start(mdlin_sb, d_mdlin)
            nc.sync.dma_start(mdlinb_row, d_mdlinb_row)
            nc.sync.dma_start(b_mdproj, d_mdprojb.rearrange("(a p) -> p a", p=128))
            nc.sync.dma_start(b_pdlin, d_pdlinb.rearrange("(a p) -> p a", p=128))

        # ---------------- persistent activations (per-sample reuse) ------
        ph2 = [apool.tile([128, 3 * 128], bf16, name=f"ph2_{c}") for c in range(EB)]
        phT = [apool.tile([128, E], bf16, name=f"phT{i}") for i in range(len(SBLK))]
        mdx = [apool.tile([128, TP], bf16, name="mdx0")]
        logitbf = [apool.tile([128, TP], bf16, name=f"logitbf{v}") for v in range(VB)]
        pen_sb = apool.tile([1, 3 * 128], bf16, name="pen_sb")

        def zero_borders(tiles):
            for t_ in tiles:
                w = t_.shape[1]
                nc.vector.memset(t_[:, 0:1], 0.0)
                nc.vector.memset(t_[:, w - 1:w], 0.0)

        zero_borders(mdx)

        # ---------------- conv helpers ----------------
        def pair_ap(t3, b0, b1, W):
            """Moving AP for a DR pass: blocks b0=(c0,k0), b1=(c1,k1) read from
            the packed fp8 tile t3 [rows, NC, XMW] (middle-dim stride >= 2)."""
            (c0, k0), (c1, k1) = b0, b1
            a0 = t3[:, c0, k0:k0 + W]
            d = t3[:, c1, k1:k1 + 1].offset - t3[:, c0, k0:k0 + 1].offset
            return bass.AP(tensor=a0.tensor, offset=a0.offset,
                           ap=[list(a0.ap[0]), [d, 2], [1, W]])

        def wpair_ap(wt, b0, b1, h):
            """Stationary AP for a DR pass from packed weights [rows, NB, Co]."""
            i0 = b0[0] * 3 + b0[1]
            i1 = b1[0] * 3 + b1[1]
            a0 = wt[:, i0, h * 128:(h + 1) * 128]
            d = wt[:, i1, 0:1].offset - wt[:, i0, 0:1].offset
            return bass.AP(tensor=a0.tensor, offset=a0.offset,
                           ap=[list(a0.ap[0]), [d, 2], [1, 128]])

        def conv_passes(n_cin):
            """DR pass plan over blocks (c,k): (c,k0)+(c,k2) pairs (stride 2),
            then k1 blocks paired across cin (stride XMW), odd k1 single."""
            p = [('dr', (c, 0), (c, 2)) for c in range(n_cin)]
            k1 = [(c, 1) for c in range(n_cin)]
            p += [('dr', k1[2 * i], k1[2 * i + 1]) for i in range(n_cin // 2)]
            if n_cin % 2:
                p.append(('single', k1[-1], None))
            return p

        def emit_half(ps, W, n_cin, fp8, wt, h, xm, xm8, i):
            if fp8:
                passes = conv_passes(n_cin)
                for j, (kind, b0, b1) in enumerate(passes):
                    if kind == 'dr':
                        nc.tensor.matmul(ps[:, :W], wpair_ap(wt, b0, b1, h),
                                         pair_ap(xm8[i], b0, b1, W),
                                         start=(j == 0), stop=(j == len(passes) - 1),
                                         perf_mode=DR)
                    else:
                        c, k = b0
                        nc.tensor.matmul(ps[:, :W],
                                         wt[:, c * 3 + k, h * 128:(h + 1) * 128],
                                         xm8[i][:, c, k:k + W],
                                         start=(j == 0), stop=(j == len(passes) - 1))
            else:
                nmm = 3 * n_cin
                idx = 0
                split = isinstance(wt, tuple)
                hb = n_cin
                for c in range(n_cin):
                    for k in range(3):
                        b = c * 3 + k
                        wsrc = (wt[b // hb][:, b % hb, h * 128:(h + 1) * 128]
                                if split else wt[:, b, h * 128:(h + 1) * 128])
                        nc.tensor.matmul(ps[:, :W], wsrc,
                                         xm[(c, i)][:, k:k + W],
                                         start=(idx == 0), stop=(idx == nmm - 1))
                        idx += 1

        def glu_layer(x_tiles, n_cin, tiles_list, TPAD, mask_rep, g, l,
                      wa_t, wg_t, pre_chunk_hook=None, pre_tile_hook=None):
            """One masked GLU conv block, in place on x_tiles (bf16).

            g: group name (QCONF + tabs key), l: layer for table lookup.
            wa_t/wg_t: packed SBUF weight tiles [128, 3*NC, Co].
            pre_chunk_hook(i)/pre_tile_hook(i): interleave points (attention,
            logits etc. ride under the dense conv matmuls).
            """
            a8, g8 = QCONF[g]['a'][l], QCONF[g]['g'][l]
            tab = tabs[g]
            n_half = n_cin  # cout == 2*cin for every GLU here
            xm = {}
            xm8 = {}

            def emit_xm(i):
                if pre_chunk_hook is not None:
                    pre_chunk_hook(i)
                t0, W = tiles_list[i]
                cw = min(W + 2, TPAD - t0)
                for c in range(n_cin):
                    xt = xring.tile([128, XMW], bf16, tag="xm", bufs=12,
                                    name=f"xm{c}_{i}")
                    nc.vector.tensor_tensor(
                        out=xt[:, :cw], in0=x_tiles[c][:, t0:t0 + cw],
                        in1=mask_rep[:, t0:t0 + cw], op=aop.mult)
                    xm[(c, i)] = xt
                if a8 or g8:
                    x8 = xring.tile([128, n_cin, XMW], f8e4, tag="xm8", bufs=2,
                                    name=f"xm8_{i}")
                    for c in range(n_cin):
                        nc.vector.tensor_tensor(
                            out=x8[:, c, :cw], in0=x_tiles[c][:, t0:t0 + cw],
                            in1=mask_rep[:, t0:t0 + cw], op=aop.mult)
                    xm8[i] = x8

            emit_xm(0)
            for i, (t0, W) in enumerate(tiles_list):
                if i + 1 < len(tiles_list):
                    emit_xm(i + 1)
                if pre_tile_hook is not None:
                    pre_tile_hook(i)
                for h in range(n_half):
                    a_ps = ps_conv("a_ps")
                    g_ps = ps_conv("g_ps")
                    emit_half(a_ps, W, n_cin, a8, wa_t, h, xm, xm8, i)
                    emit_half(g_ps, W, n_cin, g8, wg_t, h, xm, xm8, i)
                    sig = spool.tile([128, 512], bf16, tag="sig", bufs=2, name="sig")
                    nc.scalar.activation(sig[:, :W], g_ps[:, :W], AF.Sigmoid,
                                         bias=tab[:, l, 2, h:h + 1],
                                         scale=tab[:, l, 3, h:h + 1] if g8 else 1.0)
                    tmp = spool.tile([128, 512], bf16, tag="tmp", bufs=2, name="tmp")
                    nc.vector.scalar_tensor_tensor(
                        out=tmp[:, :W], in0=a_ps[:, :W],
                        scalar=tab[:, l, 0, h:h + 1],
                        in1=sig[:, :W], op0=aop.add, op1=aop.mult)
                    # x = tmp [* sa_inv] + xm   (masked residual)
                    if a8:
                        nc.vector.scalar_tensor_tensor(
                            out=x_tiles[h][:, 1 + t0:1 + t0 + W], in0=tmp[:, :W],
                            scalar=tab[:, l, 1, h:h + 1],
                            in1=xm[(h, i)][:, 1:1 + W], op0=aop.mult, op1=aop.add)
                    else:
                        nc.vector.tensor_tensor(
                            out=x_tiles[h][:, 1 + t0:1 + t0 + W], in0=tmp[:, :W],
                            in1=xm[(h, i)][:, 1:1 + W], op=aop.add)

        def _wring_bufs(R, NB, Co, dt8):
            """Ring depth per (shape, dtype) class: max tiles concurrently
            live in one layer (a+g share classes) + 1 for prefetch."""
            n_live = 0
            for g, (L, R_, NB_, Co_) in GSHAPE.items():
                for l in range(L):
                    if g == 'me' and l == 0:
                        continue
                    live = sum(1 for h in ('a', 'g')
                               if (R_, NB_, Co_) == (R, NB, Co)
                               and (wdt(g, h, l) == f8e4) == dt8)
                    # split bf16 halves count double
                    n_live = max(n_live, live)
            return n_live + 1

        def load_w(g, l):
            """Load one layer's packed conv weights (both halves).  Ring tags
            are shared by (shape, dtype) class so me/pe/pd rotate through the
            same SBUF buffers.  Large bf16 tiles (dec 'a') split in two along
            the block dim — fine for bf16 (no DR pairing constraints)."""
            out = []
            for h in ('a', 'g'):
                dtensor = d_w[(g, h, l)]
                R, NB, Co = dtensor.shape
                dt = wdt(g, h, l)
                if dt == bf16 and NB * Co * 2 > 16384:
                    hb = NB // 3
                    wts = []
                    for p_ in range(3):
                        wt = wring.tile([R, hb, Co], dt, tag=f"w{R}x{hb}x{Co}_b",
                                        bufs=4, name=f"w_{g}_{h}_{l}_{p_}")
                        nc.sync.dma_start(wt, dtensor[:, p_ * hb:(p_ + 1) * hb, :])
                        wts.append(wt)
                    out.append(tuple(wts))
                else:
                    dt8 = dt == f8e4
                    wt = wring.tile([R, NB, Co], dt,
                                    tag=f"w{R}x{NB}x{Co}_{'8' if dt8 else 'b'}",
                                    bufs=2 if g == 'me0'
                                    else _wring_bufs(R, NB, Co, dt8),
                                    name=f"w_{g}_{h}_{l}")
                    nc.sync.dma_start(wt, dtensor)
                    out.append(wt)
            return out

        def bcast_row(row_ap):
            # [1, N] AP -> stride-0 partition broadcast AP [128, N] for DMA
            return bass.AP(tensor=row_ap.tensor, offset=row_ap.offset,
                           ap=[[0, 128]] + [list(d) for d in row_ap.ap[1:]])

        def emit_ph_phase(s2):
            """Phoneme embedding + encoder + attention prep for sample s2.
            Emitted one sample ahead (after s2-1's decoder) so the PE never
            drains at sample boundaries."""
            phx = [apool.tile([128, SP], bf16, tag=f"phx{c}", bufs=1,
                              name=f"phx{c}") for c in range(EB)]
            zero_borders(phx)
            phmask = spool.tile([128, SP], bf16, tag="phmask", bufs=1,
                                name="phmask")
            nc.sync.dma_start(out=phmask, in_=bcast_row(d_phmask[s2:s2 + 1, :]))
            for c in range(EB):
                nc.sync.dma_start(phx[c], d_embph[s2, c * 128:(c + 1) * 128, :])
            for l in range(4):
                wa_t, wg_t = load_w('pe', l)
                glu_layer(phx, EB, [(0, S)], SP, phmask, 'pe', l, wa_t, wg_t)

            # ---- attention prep: ph2 = 2*ph_enc, p2 = sum(ph^2), phT ----
            p2_ps = ps_tile("p2_ps")
            for c in range(EB):
                nc.vector.tensor_scalar_mul(ph2[c][:, :S], phx[c][:, 1:1 + S], 2.0)
                nc.vector.memset(ph2[c][:, S:], 0.0)
                sq = spool.tile([128, S], bf16, tag="sq", bufs=1, name="sq")
                nc.vector.tensor_tensor(sq[:, :S], phx[c][:, 1:1 + S],
                                        phx[c][:, 1:1 + S], op=aop.mult)
                nc.tensor.matmul(p2_ps[0:1, :S], ones_col, sq[:, :S],
                                 start=(c == 0), stop=(c == EB - 1))
            phpen_f = spool.tile([1, S], f32, tag="phpen", bufs=1, name="phpen_f")
            nc.sync.dma_start(phpen_f, d_phpen[s2:s2 + 1, :])
            # pen = -p2 + phpen   (phpen is 0 valid / -1e9 masked)
            nc.vector.scalar_tensor_tensor(
                out=pen_sb[0:1, :S], in0=p2_ps[0:1, :S], scalar=-1.0,
                in1=phpen_f[0:1, :S], op0=aop.mult, op1=aop.add)
            nc.vector.memset(pen_sb[0:1, S:], -1e9)
            for c in range(EB):
                for si, (s0, sw) in enumerate(SBLK):
                    tr = ps_tile_bf("trph_ps")
                    nc.tensor.transpose(tr[:sw, :128],
                                        phx[c][:, 1 + s0:1 + s0 + sw],
                                        ident)
                    nc.vector.tensor_copy(phT[si][:sw, c * 128:(c + 1) * 128],
                                          tr[:sw, :128])

        # ================ per-sample pipeline ================
        for s in range(ns):
            # beyond mel_len the masked convs see zeros, so everything is a
            # per-channel constant: compute t < Teff only, broadcast the tail
            Teff = teffs[s]
            TTs = _ttiles(Teff)
            TBLKs = _ttiles(Teff, 128)
            if s == 0:
                load_consts_early()
            emit_ph_phase(s)
            if s == 0:
                load_consts_late()
            decx = [apool.tile([128, TP], bf16, tag=f"decx{c}",
                               bufs=2 if c < EB else 1, name=f"decx{c}")
                    for c in range(HB)]
            zero_borders(decx)
            if s < 2 and Teff < T:
                for t_ in decx + mdx + logitbf:
                    nc.vector.memset(t_[:, 1 + Teff:TP - 1], 0.0)
            # ---- mel mask (partition-broadcast via stride-0 DMA) ----
            melmask = spool.tile([128, TP], bf16, tag="melmask", bufs=1,
                                 name="melmask")
            nc.sync.dma_start(out=melmask, in_=bcast_row(d_melmask[s:s + 1, :]))

            # ---- masked mels tape [mels*mask; mask] for the fused proj/L0;
            # row 80 of the dram mels is ones, so one mask-multiply fills
            # both the masked mels and the mask row ----
            mels_m = spool.tile([MEL_DIMS + 1, TP], bf16, tag="melsm", bufs=1,
                                name="mels_m")
            nc.sync.dma_start(mels_m[:, 1:1 + Teff], d_mels[s, :, :Teff])
            nc.vector.memset(mels_m[:, 0:1], 0.0)
            nc.vector.memset(mels_m[:, TP - 1:TP], 0.0)

            def mel0_layer():
                """Fused mel projection + first GLU layer: conv contraction is
                the 81-row masked-mels tape (w_me0 = P81 @ me_W[0]); the proj
                output (= masked residual) comes from the same tape."""
                wa0, wg0 = load_w('me0', 0)
                a8, g8 = QCONF['me0']['a'][0], QCONF['me0']['g'][0]
                tab = tabs['me']
                x0 = {}
                t8 = {}

                def emit_tape8(i):
                    # cast tile i's tape to fp8 -- callable only once tile
                    # i+1's mask-mult has filled the right halo columns
                    t0, W = TTs[i]
                    cw = min(W + 2, TP - t0)
                    x8 = xring.tile([MEL_DIMS + 1, 1, XMW], f8e4, tag="tape8",
                                    bufs=3, name=f"tape8_{i}")
                    nc.vector.tensor_copy(x8[:, 0, :cw],
                                          mels_m[0:MEL_DIMS + 1, t0:t0 + cw])
                    t8[i] = x8

                def emit_proj(i):
                    t0, W = TTs[i]
                    nc.vector.tensor_tensor(
                        out=mels_m[0:MEL_DIMS + 1, 1 + t0:1 + t0 + W],
                        in0=mels_m[0:MEL_DIMS + 1, 1 + t0:1 + t0 + W],
                        in1=melmask[0:MEL_DIMS + 1, 1 + t0:1 + t0 + W],
                        op=aop.mult)
                    if i == len(TTs) - 1 and Teff < T:
                        nc.vector.memset(
                            mels_m[:, 1 + Teff:min(TP, 3 + Teff)], 0.0)
                    if a8 or g8:
                        if i > 0:
                            emit_tape8(i - 1)
                        if i == len(TTs) - 1:
                            emit_tape8(i)
                    for eb in range(EB):
                        ps = ps_conv("proj_ps")
                        nc.tensor.matmul(ps[:, :W],
                                         meproj81_sb[:, eb * 128:(eb + 1) * 128],
                                         mels_m[0:MEL_DIMS + 1, 1 + t0:1 + t0 + W],
                                         start=True, stop=True)
                        xt = spool.tile([128, 512], bf16, tag="x0", bufs=6,
                                        name=f"x0_{eb}")
                        nc.vector.tensor_copy(xt[:, :W], ps[:, :W])
                        x0[(eb, i)] = xt

                def emit_half0(ps, W, i, t0, fp8, wt, h):
                    if fp8:
                        nc.tensor.matmul(ps[:, :W], wpair_ap(wt, (0, 0), (0, 2), h),
                                         pair_ap(t8[i], (0, 0), (0, 2), W),
                                         start=True, stop=False, perf_mode=DR)
                        nc.tensor.matmul(ps[:, :W],
                                         wt[:, 1, h * 128:(h + 1) * 128],
                                         t8[i][:, 0, 1:1 + W],
                                         start=False, stop=True)
                    else:
                        for k in range(3):
                            nc.tensor.matmul(
                                ps[:, :W], wt[:, k, h * 128:(h + 1) * 128],
                                mels_m[0:MEL_DIMS + 1, t0 + k:t0 + k + W],
                                start=(k == 0), stop=(k == 2))

                emit_proj(0)
                for i, (t0, W) in enumerate(TTs):
                    if i + 1 < len(TTs):
                        emit_proj(i + 1)
                    for h in range(EB):
                        a_ps = ps_conv("a_ps")
                        g_ps = ps_conv("g_ps")
                        emit_half0(a_ps, W, i, t0, a8, wa0, h)
                        emit_half0(g_ps, W, i, t0, g8, wg0, h)
                        sig = spool.tile([128, 512], bf16, tag="sig", bufs=2,
                                         name="sig")
                        nc.scalar.activation(sig[:, :W], g_ps[:, :W], AF.Sigmoid,
                                             bias=tab[:, 0, 2, h:h + 1],
                                             scale=tab[:, 0, 3, h:h + 1] if g8 else 1.0)
                        tmp = spool.tile([128, 512], bf16, tag="tmp", bufs=2,
                                         name="tmp")
                        nc.vector.scalar_tensor_tensor(
                            out=tmp[:, :W], in0=a_ps[:, :W],
                            scalar=tab[:, 0, 0, h:h + 1],
                            in1=sig[:, :W], op0=aop.add, op1=aop.mult)
                        if a8:
                            nc.vector.scalar_tensor_tensor(
                                out=decx[h][:, 1 + t0:1 + t0 + W], in0=tmp[:, :W],
                                scalar=tab[:, 0, 1, h:h + 1],
                                in1=x0[(h, i)][:, :W], op0=aop.mult, op1=aop.add)
                        else:
                            nc.vector.tensor_tensor(
                                out=decx[h][:, 1 + t0:1 + t0 + W], in0=tmp[:, :W],
                                in1=x0[(h, i)][:, :W], op=aop.add)

            def attn_group(gi, s=s, decx=decx):
                g0, GW = TTs[gi]
                atg = [spool.tile([128, 512], bf16, tag=f"attnT{si}", bufs=1,
                                  name=f"attnT{si}") for si in range(len(SBLK))]

                def transposes(pend):
                    # deferred one block so the PE rides the next block's
                    # scores while this block's softmax finishes
                    TWp, jp, attnp = pend
                    for si, (s0, sw) in enumerate(SBLK):
                        tr = ps_tile_bf("trat_ps")
                        nc.tensor.transpose(tr[:sw, :TWp],
                                            attnp[:TWp, s0:s0 + sw],
                                            ident[:TWp, :TWp])
                        nc.vector.tensor_copy(atg[si][:sw, jp * 128:jp * 128 + TWp],
                                              tr[:sw, :TWp])

                pend = None
                for t0 in range(g0, g0 + GW, 128):
                    TW = min(128, g0 + GW - t0)
                    j = (t0 - g0) // 128
                    s_ps = ps_tile("s_ps")
                    for c in range(EB):
                        nc.tensor.matmul(s_ps[:TW, :S],
                                         decx[c][:, 1 + t0:1 + t0 + TW],
                                         ph2[c][:, :S], start=(c == 0), stop=False)
                    nc.tensor.matmul(s_ps[:TW, :S], ones_row[:, :TW],
                                     pen_sb[0:1, :S], start=False, stop=True)
                    negmx = spool.tile([128, 1], f32, tag="negmx", bufs=4, name="negmx")
                    nc.vector.reduce_max(negmx[:TW], s_ps[:TW, :S], axis=AX.X,
                                         negate=True)
                    attn = spool.tile([128, S], bf16, tag="attn", bufs=2, name="attn")
                    sumexp = spool.tile([128, 1], f32, tag="sumexp", bufs=4,
                                        name="sumexp")
                    nc.scalar.activation(attn[:TW, :S], s_ps[:TW, :S], AF.Exp,
                                         bias=negmx[:TW], scale=1.0,
                                         accum_out=sumexp[:TW])
                    rcp = spool.tile([128, 1], f32, tag="rcp", bufs=4, name="rcp")
                    nc.vector.reciprocal(rcp[:TW], sumexp[:TW])
                    nc.vector.tensor_scalar_mul(attn[:TW, :S], attn[:TW, :S],
                                                rcp[:TW])
                    if pend is not None:
                        transposes(pend)
                    pend = (TW, j, attn)
                transposes(pend)
                for eb in range(EB):
                    ctx = ps_tile("ctx_ps")
                    for si, (s0, sw) in enumerate(SBLK):
                        nc.tensor.matmul(ctx[:, :GW],
                                         phT[si][:sw, eb * 128:(eb + 1) * 128],
                                         atg[si][:sw, :GW],
                                         start=(si == 0), stop=(si == len(SBLK) - 1))
                    nc.vector.tensor_copy(decx[EB + eb][:, 1 + g0:1 + g0 + GW],
                                          ctx[:, :GW])

            # ---- mel encoder: 4 GLU layers on decx[0:3]; attention group g
            # is emitted under layer 3's dense convs right after the tile
            # that produces its mel_enc columns ----
            def mel3_hook(i):
                if i >= 1:
                    attn_group(i - 1)

            mel0_layer()
            for l in range(1, 4):
                wa_t, wg_t = load_w('me', l)
                glu_layer(decx[:EB], EB, TTs, TP, melmask, 'me', l, wa_t, wg_t,
                          pre_tile_hook=mel3_hook if l == 3 else None)

            def dec0_chunk_hook(i):
                # the last attention group rides under dec L0's dense convs.
                # It must precede dec L0's xm for tile last-1, whose right
                # halo column reads the first ctx column of the last tile.
                if i == max(0, len(TTs) - 2):
                    attn_group(len(TTs) - 1)

            # ---- logits (feature-major) -> logitbf, then mel_h0 ----
            def lgprep(ti):
                t0, W = TTs[ti]
                for vb in range(VB):
                    lf = ps_tile("lf_ps")
                    for c in range(HB):
                        nc.tensor.matmul(lf[:, :W],
                                         pdlin_sb[:, c, vb * 128:(vb + 1) * 128],
                                         decx[c][:, 1 + t0:1 + t0 + W],
                                         start=(c == 0), stop=(c == HB - 1))
                    nc.vector.tensor_scalar_add(logitbf[vb][:, 1 + t0:1 + t0 + W],
                                                lf[:, :W], b_pdlin[:, vb:vb + 1])
                mh = ps_tile("mh_ps")
                for vb in range(VB):
                    nc.tensor.matmul(mh[:, :W], mdproj_sb[:, vb, :],
                                     logitbf[vb][:, 1 + t0:1 + t0 + W],
                                     start=(vb == 0), stop=(vb == VB - 1))
                nc.vector.tensor_scalar_add(mdx[0][:, 1 + t0:1 + t0 + W],
                                            mh[:, :W], b_mdproj[:, 0:1])

            # ---- log_softmax tiles (emitted interleaved with conv work).
            # One Ln per 512-tile (4 blocks) keeps the scalar engine from
            # thrashing activation tables between Exp and Ln. ----
            nblk = len(TBLKs)
            outlps = []

            def lg_tile(ti, s=s, outlps=outlps):
                blocks = list(blocks_of(ti))
                se2a = spool.tile([128, 4], f32, tag="se2a", bufs=2, name="se2a")
                pend = []
                for j, tb in enumerate(blocks):
                    t0, TW = TBLKs[tb]
                    lg = ps_tile_bf("lg_ps")
                    for vb in range(VB):
                        nc.tensor.transpose(lg[:TW, vb * 128:(vb + 1) * 128],
                                            logitbf[vb][:, 1 + t0:1 + t0 + TW],
                                            ident)
                    negmx2 = spool.tile([128, 1], f32, tag="negmx2", bufs=5,
                                        name="negmx2")
                    nc.vector.reduce_max(negmx2[:TW], lg[:TW, :V], axis=AX.X,
                                         negate=True)
                    esc = spool.tile([128, V], bf16, tag="esc", bufs=1, name="esc")
                    nc.scalar.activation(esc[:TW, :V], lg[:TW, :V], AF.Exp,
                                         bias=negmx2[:TW], scale=1.0,
                                         accum_out=se2a[:TW, j:j + 1])
                    outlp = spool.tile([128, V], f32, tag="outlp", bufs=5,
                                       name="outlp")
                    nc.vector.tensor_scalar_add(outlp[:TW, :V], lg[:TW, :V],
                                                negmx2[:TW])
                    pend.append((tb, t0, TW, outlp))
                lna = spool.tile([128, 4], f32, tag="lna", bufs=2, name="lna")
                nc.scalar.activation(lna[:, :len(blocks)], se2a[:, :len(blocks)],
                                     AF.Ln)
                for j, (tb, t0, TW, outlp) in enumerate(pend):
                    nc.vector.tensor_scalar_sub(outlp[:TW, :V], outlp[:TW, :V],
                                                lna[:TW, j:j + 1])
                    nc.sync.dma_start(d_out[s, t0:t0 + TW, 0:V], outlp[:TW, :V])
                    outlps.append(outlp)

            def preds_block(tb, s=s, mdx=mdx):
                t0, TW = TBLKs[tb]
                mp = ps_tile("mp_ps")
                nc.tensor.matmul(mp[:TW, :MEL_DIMS], mdx[0][:, 1 + t0:1 + t0 + TW],
                                 mdlin_sb[:, :MEL_DIMS], start=True, stop=False)
                nc.tensor.matmul(mp[:TW, :MEL_DIMS], ones_row[:, :TW], mdlinb_row,
                                 start=False, stop=True)
                outmp = spool.tile([128, MEL_DIMS], f32, tag="outmp", bufs=2,
                                   name="outmp")
                s2 = spool.tile([128, MEL_DIMS], f32, tag="s2", bufs=2, name="s2")
                nc.scalar.activation(s2[:TW, :MEL_DIMS], mp[:TW, :MEL_DIMS],
                                     AF.Sigmoid, scale=2.0)
                nc.vector.tensor_scalar(out=outmp[:TW, :MEL_DIMS],
                                        in0=s2[:TW, :MEL_DIMS], scalar1=2.0,
                                        scalar2=-1.0, op0=aop.mult, op1=aop.add)
                nc.sync.dma_start(d_out[s, t0:t0 + TW, V:V + MEL_DIMS],
                                  outmp[:TW, :MEL_DIMS])
                outmps[tb] = outmp

            outmps = {}
            blocks_of = lambda i: range(4 * i, min(4 * i + 4, nblk))

            def dec1_hook(i):
                # tile i-1 of dec output is final: emit its logits work under
                # this tile's dense conv matmuls
                if i >= 1:
                    lgprep(i - 1)
                    lg_tile(i - 1)

            # ---- decoder: 2 GLU layers on decx[0:6] ----
            for l in range(2):
                wa_t, wg_t = load_w('pd', l)
                glu_layer(decx, HB, TTs, TP, melmask, 'pd', l, wa_t, wg_t,
                          pre_tile_hook=dec1_hook if l == 1 else None,
                          pre_chunk_hook=dec0_chunk_hook if l == 0 else None)
            lgprep(len(TTs) - 1)
            lg_tile(len(TTs) - 1)

            def md_hook1(i):
                if i > 0:
                    for tb in blocks_of(i - 1):
                        preds_block(tb)

            # ---- mel decoder: 2 GLU layers on mdx, sparse phases woven in ----
            for l in range(2):
                wa_t, wg_t = load_w('md', l)
                glu_layer(mdx, 1, TTs, TP, melmask, 'md', l, wa_t, wg_t,
                          pre_tile_hook=md_hook1 if l == 1 else None)
            for tb in blocks_of(len(TTs) - 1):
                preds_block(tb)

            if Teff < T:
                # broadcast the constant row Teff-1 over the skipped tail:
                # replicate it across partitions, then tile DMAs of <=128 rows
                row = TBLKs[-1][1] - 1
                rowcat = spool.tile([128, V + MEL_DIMS], f32, tag="rowcat",
                                    bufs=1, name="rowcat")
                nc.sync.dma_start(rowcat[0:1, 0:V], outlps[-1][row:row + 1, :V])
                nc.sync.dma_start(rowcat[0:1, V:V + MEL_DIMS],
                                  outmps[len(TBLKs) - 1][row:row + 1, :MEL_DIMS])
                nc.gpsimd.partition_broadcast(rowcat[:, :], rowcat[0:1, :])
                for r0 in range(Teff, T, 128):
                    rw = min(128, T - r0)
                    nc.sync.dma_start(d_out[s, r0:r0 + rw, :], rowcat[:rw, :])

        psB.release()
        psA.release()
        spool.release()
        xring.release()
        apool.release()
        wring.release()
        cpool.release()

    nc.compile()
    return nc


def plan_slots(mel_lens, ns=NS, T=T_MEL, n_cores=N_CORES, margin=12):
    """Sort samples by length; slot j of every core gets rank 8j+core.
    Returns (order, teffs): order[core*ns + slot] = original sample index,
    teffs[slot] = compile-time effective length for that slot (same on all
    cores, so a single SPMD NEFF serves all 8)."""
    mel_lens = np.asarray(mel_lens).astype(np.int64)
    idx = np.argsort(-mel_lens, kind='stable')
    order = np.empty(ns * n_cores, np.int64)
    teffs = []
    for j in range(ns):
        grp = idx[j * n_cores:(j + 1) * n_cores]
        for c in range(n_cores):
            order[c * ns + j] = grp[c]
        te = int(mel_lens[grp].max()) + margin
        te = min(T, ((te + 127) // 128) * 128)
        teffs.append(te)
    return order, tuple(teffs)


def _pack_conv(w, q):
    """w: [3, C, Co] f32 one-layer master -> packed [R, 3*NC, Co] (+ scale).

    Block b = c*3 + k.  q=True: per-cout-channel scale s_j = 224/absmax,
    values clipped to +-240 and stored fp8; else bf16."""
    K, C, Co = w.shape
    if C % 128 == 0:
        R, NC = 128, C // 128
    else:
        R, NC = C, 1
    if q:
        am = np.abs(w).reshape(-1, Co).max(0)
        s = (224.0 / np.maximum(am, 1e-9)).astype(np.float32)
        wq = np.clip(w * s[None, None, :], -240.0, 240.0)
    else:
        s, wq = None, w
    arr = wq.reshape(3, NC, R, Co).transpose(2, 1, 0, 3)
    arr = np.ascontiguousarray(arr.reshape(R, 3 * NC, Co))
    return arr.astype(F8 if q else BF), s


def _tab_row(b, Co, sa, sg):
    """One layer's bias/scale table [4, Co]: (b_a_eff, sa_inv, b_g, sg_inv)."""
    t = np.zeros((4, Co), np.float32)
    ba, bg = b[:Co], b[Co:]
    t[0] = ba * (sa if sa is not None else 1.0)
    t[1] = (1.0 / sa) if sa is not None else 1.0
    t[2] = bg
    t[3] = (1.0 / sg) if sg is not None else 1.0
    return t


def preprocess(inputs, ns=NS, T=T_MEL, TPH=T_PHON, n_cores=N_CORES, order=None):
    """Host-side prep: transpose/pad/cast, build masks, pack weights, shard."""
    S = TPH + 1
    TP = T + 2
    SP = S + 2
    B = ns * n_cores

    mels = np.asarray(inputs['mels'], np.float32)[:B, :T]
    phonemes = np.asarray(inputs['phonemes']).astype(np.int64)[:B, :TPH]
    mel_lens = np.asarray(inputs['mel_lens']).astype(np.int64)[:B]
    phoneme_lens = np.asarray(inputs['phoneme_lens']).astype(np.int64)[:B]
    if order is not None:
        mels = mels[order]
        phonemes = phonemes[order]
        mel_lens = mel_lens[order]
        phoneme_lens = phoneme_lens[order]
    emb = np.asarray(inputs['emb'], np.float32)

    mels_t = np.concatenate(
        [mels.transpose(0, 2, 1), np.ones((B, 1, T), np.float32)],
        axis=1).astype(BF)  # [B, 81, T]; row 80 = ones (mask source)

    ph = np.concatenate([np.zeros((B, 1), np.int64), phonemes], axis=1)  # [B,S]
    embph = emb[ph]                                    # [B, S, E] f32
    embph_t = np.zeros((B, E, SP), np.float32)
    embph_t[:, :, 1:1 + S] = embph.transpose(0, 2, 1)
    embph_t = embph_t.astype(BF)

    t_idx = np.arange(T)
    melmask = np.zeros((B, TP), np.float32)
    melmask[:, 1:1 + T] = (t_idx[None, :] < mel_lens[:, None]).astype(np.float32)
    melmask = melmask.astype(BF)

    s_idx = np.arange(S)
    ph_valid = s_idx[None, :] <= phoneme_lens[:, None]
    phmask = np.zeros((B, SP), np.float32)
    phmask[:, 1:1 + S] = ph_valid.astype(np.float32)
    phmask = phmask.astype(BF)
    phpen = np.where(ph_valid, 0.0, -1e9).astype(np.float32)  # [B, S]

    # fold the mel projection into the first mel GLU layer: with the mask
    # applied to raw mels, conv(proj(mels)*mask) == (mels*mask) @ (P @ W0_k),
    # and the proj bias contributes exactly b@W0_k per position times the
    # mask value -- append the mask itself as input row 80 (P' row 80 = b).
    P81 = np.concatenate([np.asarray(inputs['me_proj_W'], np.float64),
                          np.asarray(inputs['me_proj_b'], np.float64)[None]], 0)
    me_w0 = np.einsum('ce,kef->kcf', P81,
                      np.asarray(inputs['me_W'], np.float64)[0]).astype(np.float32)

    shared = {
        'me_proj81': P81.astype(np.float32).astype(BF),
        'pd_lin': np.asarray(inputs['pd_lin_W'], np.float32).astype(BF),
        'md_proj': np.asarray(inputs['md_proj_W'], np.float32).astype(BF),
        'md_lin': np.asarray(inputs['md_lin_W'], np.float32).astype(BF),
        'md_lin_b_row': np.asarray(inputs['md_lin_b'], np.float32)[None, :].astype(BF),
        'md_proj_b': np.asarray(inputs['md_proj_b'], np.float32),
        'pd_lin_b': np.asarray(inputs['pd_lin_b'], np.float32),
    }

    # packed conv weights + tables
    masters = {
        'me0': (me_w0[None], None),
        'me': (np.asarray(inputs['me_W'], np.float32), 'me_b'),
        'pe': (np.asarray(inputs['pe_W'], np.float32), 'pe_b'),
        'pd': (np.asarray(inputs['pd_W'], np.float32), 'pd_b'),
        'md': (np.asarray(inputs['md_W'], np.float32), 'md_b'),
    }
    scales = {}
    for g, (w, _) in masters.items():
        Co = w.shape[-1] // 2
        for h, sl in (('a', slice(0, Co)), ('g', slice(Co, None))):
            for l in range(w.shape[0]):
                if g == 'me' and l == 0:
                    continue
                arr, s = _pack_conv(np.ascontiguousarray(w[l, ..., sl]),
                                    QCONF[g][h][l])
                shared[f'w_{g}_{h}_{l}'] = arr
                scales[(g, h, l)] = s
    for g in ('me', 'pe', 'pd', 'md'):
        b = np.asarray(inputs[masters[g][1]], np.float32)
        L = b.shape[0]
        Co = b.shape[-1] // 2
        t = np.zeros((L, 4, Co), np.float32)
        for l in range(L):
            if g == 'me' and l == 0:
                # layer 0 of the mel encoder is the fused me0 conv
                t[0] = _tab_row(b[0], Co, scales[('me0', 'a', 0)],
                                scales[('me0', 'g', 0)])
            else:
                t[l] = _tab_row(b[l], Co, scales[(g, 'a', l)], scales[(g, 'g', l)])
        nh = Co // 128
        tp = t.reshape(L, 4, nh, 128).transpose(3, 0, 1, 2)
        shared[f'tab_{g}'] = np.ascontiguousarray(tp)

    in_maps = []
    for core in range(n_cores):
        sl = slice(core * ns, (core + 1) * ns)
        m = dict(shared)
        m['mels'] = np.ascontiguousarray(mels_t[sl])
        m['embph'] = np.ascontiguousarray(embph_t[sl])
        m['melmask'] = np.ascontiguousarray(melmask[sl])
        m['phmask'] = np.ascontiguousarray(phmask[sl])
        m['phpen'] = np.ascontiguousarray(phpen[sl])
        in_maps.append(m)
    return in_maps


_CACHE = {}


def _get_nc(teffs=None):
    key = teffs if teffs is not None else ('full',)
    if key not in _CACHE:
        _CACHE[key] = build(teffs=list(teffs) if teffs is not None else None)
    return _CACHE[key]


def kernel(**inputs) -> np.ndarray:
    from concourse.bass_utils import run_bass_kernel_spmd
    order, teffs = plan_slots(np.asarray(inputs['mel_lens']))
    nc = _get_nc(teffs)
    in_maps = preprocess(inputs, order=order)
    res = run_bass_kernel_spmd(nc, in_maps, core_ids=list(range(N_CORES)))
    out = np.concatenate([r['out'] for r in res.results], axis=0)
    inv = np.empty_like(order)
    inv[order] = np.arange(len(order))
    out = out[inv]
    return np.ascontiguousarray(out.astype(np.float32))


if __name__ == '__main__':
    import reference
    inputs = {k: np.asarray(v) for k, v in reference.setup_inputs().items()}
    out = kernel(**inputs)
    print(out.shape, out.dtype)
